# revision 3
# baseline (speedup 1.0000x reference)
"""Trainium2 Bass kernel for nn_C3_layer (dense 5x5 VALID conv, 6->16 channels).

Full input x [32,6,512,512] f32 -> full output [32,16,508,508] f32.
Data-parallel over batch: 4 images per core across 8 NeuronCores.

v2 design (width-packed bf16 block-Toeplitz conv-as-matmul):
  - All device I/O in bf16 (PSUM accumulation stays f32): host pre-casts x
    and weights, post-casts y. Halves HBM traffic vs f32; rel-err ~3e-3,
    well inside the 2e-2 gate.
  - Width packing S=2: host de-interleaves x columns into
    xp[b,ci,h,s,j] = x[b,ci,h,2j+s]. One matmul column then carries TWO
    output pixels, so a 4-output-row block needs only 3 matmuls of N=254
    (vs 5 of N=508 for the unpacked R=8 layout): per-block PE time
    3*254 = 762 cyc vs 2540 cyc -- TensorE drops ~1.67x. bf16 operands are
    required for 1 cyc/row at N<256 (fp32r degrades to 4 cyc/row there).
  - Block = 4 output rows: psum[m=(co,r,p), j] with M=16*4*2=128,
    contraction k=(i,ci,s), K=8*6*2=96, weights T[d][k,m] = W[co,ci,i-r,kw]
    with kw=2d+s-p (valid taps only), moving slice xt[:, d:d+254].
  - No halo reuse: each block freshly DMAs its 96x512B input tile. The v1
    kernel's chained SBUF->SBUF halo copies serialized on the blocking
    gpsimd queue (~252 hops x ~1.9us ~= the whole 488us baseline runtime);
    re-reading costs only ~12.6MB/core extra HBM (~35us) and has no
    cross-block dependency, so all DMAs pipeline.
  - Out rows are packed the same way: y[b,co,oh,p,j], host re-interleaves.
    Evac+bias on DVE (f32 psum -> bf16 SBUF), out-DMA issued by ACT (second
    HWDGE ring), in-DMA by SP.
"""

import os

import numpy as np

KK = 5    # conv kernel size
R = 4     # output rows per block
S = 2     # width packing factor
ND = 3    # number of moving-shift matmuls per block (kw+p = 2d+s)
B_PER_CORE = 4
N_CORES = 8
H = 512
W = 512
HO = H - 4
WO = W - 4
WP = W // S    # 256 packed input columns
WOP = WO // S  # 254 packed output columns
KDIM = (R + KK - 1) * 6 * S  # 96 contraction rows
MDIM = 16 * R * S            # 128 output partitions
NBLK = HO // R               # 127 blocks per image

CH3 = np.array([[0, 1, 2], [1, 2, 3], [2, 3, 4], [3, 4, 5], [0, 4, 5], [0, 1, 5]])
CH4 = np.array([[0, 1, 2, 3], [1, 2, 3, 4], [2, 3, 4, 5], [0, 3, 4, 5], [0, 1, 4, 5],
                [0, 1, 2, 5], [0, 1, 3, 4], [1, 2, 4, 5], [0, 2, 3, 5]])

# stash of the last BassKernelResults (for test.py profiling)
LAST_RESULTS = None


def _build_full_kernel(w3, w4, w6):
    Wf = np.zeros((16, 6, KK, KK), dtype=np.float32)
    Wf[np.arange(6)[:, None], CH3] = w3
    Wf[(6 + np.arange(9))[:, None], CH4] = w4
    Wf[15] = w6[0]
    return Wf


def _build_toeplitz_packed(Wf):
    """T [ND, KDIM, MDIM]: T[d, i*12+ci*2+s, co*8+r*2+p] = Wf[co,ci,i-r,2d+s-p]
    for valid taps (0<=i-r<KK, 0<=2d+s-p<KK), else 0."""
    T = np.zeros((ND, KDIM, MDIM), dtype=np.float32)
    for d in range(ND):
        for s in range(S):
            for p in range(S):
                kw = S * d + s - p
                if not (0 <= kw < KK):
                    continue
                for r in range(R):
                    for kh in range(KK):
                        i = r + kh
                        for ci in range(6):
                            k = i * 12 + ci * 2 + s
                            m0 = r * 2 + p
                            T[d, k, m0::R * S] = Wf[:, ci, kh, kw]
    return T


def _build_bass():
    import contextlib

    import concourse.bacc as bacc
    import concourse.mybir as mybir
    import concourse.tile as tile

    f32 = mybir.dt.float32
    bf16 = mybir.dt.bfloat16
    # benchmarking only: repeat the whole conv body L times inside the NEFF
    loop_n = int(os.environ.get("CONV_BENCH_LOOP", "1"))
    in_bufs = int(os.environ.get("CONV_IN_BUFS", "16"))
    out_bufs = int(os.environ.get("CONV_OUT_BUFS", "12"))
    psum_bufs = int(os.environ.get("CONV_PSUM_BUFS", "8"))

    nc = bacc.Bacc(name="conv5x5p2")
    x = nc.dram_tensor("x", [B_PER_CORE, 6, H, S, WP], bf16,
                       kind="ExternalInput")
    t = nc.dram_tensor("t", [ND, KDIM, MDIM], bf16, kind="ExternalInput")
    bias = nc.dram_tensor("bias", [MDIM, 1], f32, kind="ExternalInput")
    y = nc.dram_tensor("y", [B_PER_CORE, 16, HO, S, WOP], bf16,
                       kind="ExternalOutput")

    with tile.TileContext(nc) as tc:
        with (
            tc.tile_pool(name="const", bufs=1) as const_pool,
            tc.tile_pool(name="xin", bufs=in_bufs) as in_pool,
            tc.tile_pool(name="yout", bufs=out_bufs) as out_pool,
            tc.tile_pool(name="psum", bufs=psum_bufs, space="PSUM") as psum_pool,
        ):
            tw = const_pool.tile([KDIM, ND * MDIM], bf16, name="tw")
            nc.sync.dma_start(out=tw[:, :], in_=t.rearrange("d k m -> k d m"))
            bias_sb = const_pool.tile([MDIM, 1], f32, name="bias_sb")
            nc.sync.dma_start(out=bias_sb[:, :], in_=bias[:, :])

            loop_cm = (tc.For_i(0, loop_n, 1) if loop_n > 1
                       else contextlib.nullcontext())
            with loop_cm:
                _emit_conv_body(nc, x, y, tw, bias_sb,
                                in_pool, out_pool, psum_pool, bf16, f32)
    nc.finalize()
    return nc


def _emit_conv_body(nc, x, y, tw, bias_sb, in_pool, out_pool, psum_pool,
                    bf16, f32):
    for b in range(B_PER_CORE):
        for blk in range(NBLK):
            h0 = R * blk
            xt = in_pool.tile([KDIM, WP], bf16, name="xt", tag="xt")
            nc.sync.dma_start(
                out=xt[:, :],
                in_=x[b, :, h0:h0 + R + KK - 1, :, :].rearrange(
                    "c h s w -> h c s w"),
            )
            ps = psum_pool.tile([MDIM, WOP], f32, name="ps", tag="ps")
            for d in range(ND):
                nc.tensor.matmul(
                    ps[:, :],
                    tw[:, d * MDIM:(d + 1) * MDIM],
                    xt[:, d:d + WOP],
                    start=(d == 0),
                    stop=(d == ND - 1),
                )
            ot = out_pool.tile([MDIM, WOP], bf16, name="ot", tag="ot")
            nc.vector.tensor_scalar_add(ot[:, :], ps[:, :], bias_sb[:, :])
            nc.scalar.dma_start(
                out=y[b, :, h0:h0 + R, :, :],
                in_=ot[:, :],
            )


def _prep_in_maps(x, w3, b3, w4, b4, w6, b6):
    from ml_dtypes import bfloat16

    x = np.asarray(x, dtype=np.float32)
    # de-interleave width: xp[b,ci,h,s,j] = x[b,ci,h,S*j+s]
    xp = np.ascontiguousarray(
        x.reshape(32, 6, H, WP, S).transpose(0, 1, 2, 4, 3).astype(bfloat16))
    Wf = _build_full_kernel(np.asarray(w3, dtype=np.float32),
                            np.asarray(w4, dtype=np.float32),
                            np.asarray(w6, dtype=np.float32))
    T = np.ascontiguousarray(_build_toeplitz_packed(Wf).astype(bfloat16))
    bias16 = np.concatenate([np.asarray(b3, dtype=np.float32),
                             np.asarray(b4, dtype=np.float32),
                             np.asarray(b6, dtype=np.float32)])
    bias_col = np.ascontiguousarray(
        np.repeat(bias16, R * S)[:, None], dtype=np.float32)  # [co*8+r*2+p, 1]
    return [
        {"x": xp[i * B_PER_CORE:(i + 1) * B_PER_CORE], "t": T,
         "bias": bias_col}
        for i in range(N_CORES)
    ]


def kernel(x, w3, b3, w4, b4, w6, b6):
    global LAST_RESULTS
    from concourse.bass_utils import run_bass_kernel_spmd

    in_maps = _prep_in_maps(x, w3, b3, w4, b4, w6, b6)
    nc = _build_bass()
    res = run_bass_kernel_spmd(
        nc, in_maps, core_ids=list(range(N_CORES)),
        trace=bool(int(os.environ.get("CONV_TRACE", "0"))),
    )
    LAST_RESULTS = res
    yp = np.concatenate([r["y"] for r in res.results], axis=0)
    # re-interleave: y[b,co,oh,S*j+s] = yp[b,co,oh,s,j]
    out = yp.transpose(0, 1, 2, 4, 3).reshape(32, 16, HO, WO)
    return np.ascontiguousarray(out.astype(np.float32))


# revision 9
# speedup vs baseline: 37181.6922x; 37181.6922x over previous
"""Trainium2 Bass kernel for nn_C3_layer (dense 5x5 VALID conv, 6->16 channels).

Full input x [32,6,512,512] f32 -> full output [32,16,508,508] f32.
Data-parallel over batch: 4 images per core across 8 NeuronCores.

Width-packed bf16 block-Toeplitz conv-as-matmul:
  - All device I/O in bf16 (PSUM accumulation stays f32): host pre-casts x
    and weights, post-casts y. Halves HBM traffic vs f32; rel-err ~3e-3,
    well inside the 2e-2 gate.
  - Width packing S=2: host de-interleaves x columns into
    xp[b,(h,ci,s),j] = x[b,ci,h,2j+s]. One matmul column then carries TWO
    output pixels, so a 4-output-row block needs only 3 matmuls of N=254
    (vs 5 of N=508 for the unpacked R=8 layout): per-block PE time
    3*254 = 762 cyc vs 2540 cyc -- TensorE drops ~1.67x. bf16 operands are
    required for 1 cyc/row at N<256 (fp32r degrades to 4 cyc/row there).
  - Block = 4 output rows: psum[m=(co,r,p), j] with M=16*4*2=128,
    contraction k=(i,ci,s), K=8*6*2=96, weights T[d][k,m] = W[co,ci,i-r,kw]
    with kw=2d+s-p (valid taps only), moving slice xt[:, d:d+254].
  - No halo reuse: each block freshly DMAs its 96x512B input tile
    (one contiguous 48KB run in the host layout). The v1 kernel's chained
    SBUF->SBUF halo copies serialized on the blocking gpsimd queue
    (~252 hops x ~1.9us ~= the whole 488us baseline runtime); re-reading
    costs only ~12.6MB/core extra HBM (~35us) and has no cross-block
    dependency, so all DMAs pipeline.
  - CONV_PAIR=1 (default): two adjacent blocks share one PSUM bank
    ([128,508] f32 = 2032B < 2KB): 6 matmuls per pair, ONE evac split
    between DVE (cols 0:254, block A) and ACT (cols 254:508, block B) so
    neither engine carries the whole ~134us of evacuation work, and ONE
    out-DMA per pair writing 16x8x508 fully-contiguous bf16 runs. Odd
    block 126 is handled as an unpaired tail into y2.
  - In-DMA issued by SP, out-DMA by ACT (the two HWDGE rings).
"""

import os

import numpy as np

KK = 5    # conv kernel size
R = 4     # output rows per block
S = 2     # width packing factor
ND = 3    # number of moving-shift matmuls per block (kw+p = 2d+s)
B_PER_CORE = 4
N_CORES = 8
H = 512
W = 512
HO = H - 4
WO = W - 4
WP = W // S    # 256 packed input columns
WOP = WO // S  # 254 packed output columns
KDIM = (R + KK - 1) * 6 * S  # 96 contraction rows
MDIM = 16 * R * S            # 128 output partitions
NBLK = HO // R               # 127 blocks per image
NPAIR = NBLK // 2            # 63 paired blocks (+ 1 tail block)

CH3 = np.array([[0, 1, 2], [1, 2, 3], [2, 3, 4], [3, 4, 5], [0, 4, 5], [0, 1, 5]])
CH4 = np.array([[0, 1, 2, 3], [1, 2, 3, 4], [2, 3, 4, 5], [0, 3, 4, 5], [0, 1, 4, 5],
                [0, 1, 2, 5], [0, 1, 3, 4], [1, 2, 4, 5], [0, 2, 3, 5]])

PAIR = bool(int(os.environ.get("CONV_PAIR", "1")))

# stash of the last BassKernelResults (for test.py profiling)
LAST_RESULTS = None


def _build_full_kernel(w3, w4, w6):
    Wf = np.zeros((16, 6, KK, KK), dtype=np.float32)
    Wf[np.arange(6)[:, None], CH3] = w3
    Wf[(6 + np.arange(9))[:, None], CH4] = w4
    Wf[15] = w6[0]
    return Wf


def _build_toeplitz_packed(Wf):
    """T [ND, KDIM, MDIM]: T[d, i*12+ci*2+s, co*8+r*2+p] = Wf[co,ci,i-r,2d+s-p]
    for valid taps (0<=i-r<KK, 0<=2d+s-p<KK), else 0."""
    T = np.zeros((ND, KDIM, MDIM), dtype=np.float32)
    for d in range(ND):
        for s in range(S):
            for p in range(S):
                kw = S * d + s - p
                if not (0 <= kw < KK):
                    continue
                for r in range(R):
                    for kh in range(KK):
                        i = r + kh
                        for ci in range(6):
                            k = i * 12 + ci * 2 + s
                            m0 = r * 2 + p
                            T[d, k, m0::R * S] = Wf[:, ci, kh, kw]
    return T


def _build_bass():
    import contextlib

    import concourse.bacc as bacc
    import concourse.mybir as mybir
    import concourse.tile as tile

    f32 = mybir.dt.float32
    bf16 = mybir.dt.bfloat16
    # benchmarking only: repeat the whole conv body L times inside the NEFF
    loop_n = int(os.environ.get("CONV_BENCH_LOOP", "1"))
    in_bufs = int(os.environ.get("CONV_IN_BUFS", "16"))
    out_bufs = int(os.environ.get("CONV_OUT_BUFS", "12"))
    psum_bufs = int(os.environ.get("CONV_PSUM_BUFS", "8"))

    nc = bacc.Bacc(name="conv5x5p2")
    # x host layout: [b, (h, ci, s), j] so a block's input tile is one
    # contiguous 96x512B = 48KB run.
    x = nc.dram_tensor("x", [B_PER_CORE, H * 6 * S, WP], bf16,
                       kind="ExternalInput")
    t = nc.dram_tensor("t", [ND, KDIM, MDIM], bf16, kind="ExternalInput")
    bias = nc.dram_tensor("bias", [MDIM, 1], f32, kind="ExternalInput")
    if PAIR:
        # y: [b, pair, co, (r,p), (half,j)] -- one pair's output is a single
        # fully-contiguous 16*8*508*2B = 130KB run.
        y = nc.dram_tensor("y", [B_PER_CORE, NPAIR, 16, R * S, 2 * WOP],
                           bf16, kind="ExternalOutput")
        y2 = nc.dram_tensor("y2", [B_PER_CORE, 16, R * S, WOP], bf16,
                            kind="ExternalOutput")
    else:
        # y: [b, co, (oh, p), j]
        y = nc.dram_tensor("y", [B_PER_CORE, 16, HO * S, WOP], bf16,
                           kind="ExternalOutput")
        y2 = None

    with tile.TileContext(nc) as tc:
        with (
            tc.tile_pool(name="const", bufs=1) as const_pool,
            tc.tile_pool(name="xin", bufs=in_bufs) as in_pool,
            tc.tile_pool(name="yout", bufs=out_bufs) as out_pool,
            tc.tile_pool(name="psum", bufs=psum_bufs, space="PSUM") as psum_pool,
        ):
            tw = const_pool.tile([KDIM, ND * MDIM], bf16, name="tw")
            nc.sync.dma_start(out=tw[:, :], in_=t.rearrange("d k m -> k d m"))
            bias_sb = const_pool.tile([MDIM, 1], f32, name="bias_sb")
            nc.sync.dma_start(out=bias_sb[:, :], in_=bias[:, :])

            loop_cm = (tc.For_i(0, loop_n, 1) if loop_n > 1
                       else contextlib.nullcontext())
            with loop_cm:
                if PAIR:
                    _emit_pair_body(nc, mybir, x, y, y2, tw, bias_sb,
                                    in_pool, out_pool, psum_pool, bf16, f32)
                else:
                    _emit_conv_body(nc, x, y, tw, bias_sb,
                                    in_pool, out_pool, psum_pool, bf16, f32)
    nc.finalize()
    return nc


def _emit_conv_body(nc, x, y, tw, bias_sb, in_pool, out_pool, psum_pool,
                    bf16, f32):
    for b in range(B_PER_CORE):
        for blk in range(NBLK):
            h0 = R * blk
            xt = in_pool.tile([KDIM, WP], bf16, name="xt", tag="xt")
            nc.sync.dma_start(
                out=xt[:, :],
                in_=x[b, h0 * 12:(h0 + R + KK - 1) * 12, :],
            )
            ps = psum_pool.tile([MDIM, WOP], f32, name="ps", tag="ps")
            for d in range(ND):
                nc.tensor.matmul(
                    ps[:, :],
                    tw[:, d * MDIM:(d + 1) * MDIM],
                    xt[:, d:d + WOP],
                    start=(d == 0),
                    stop=(d == ND - 1),
                )
            ot = out_pool.tile([MDIM, WOP], bf16, name="ot", tag="ot")
            nc.vector.tensor_scalar_add(ot[:, :], ps[:, :], bias_sb[:, :])
            nc.scalar.dma_start(
                out=y[b, :, h0 * S:(h0 + R) * S, :],
                in_=ot[:, :],
            )


def _emit_pair_body(nc, mybir, x, y, y2, tw, bias_sb, in_pool, out_pool,
                    psum_pool, bf16, f32):
    Ident = mybir.ActivationFunctionType.Identity
    for b in range(B_PER_CORE):
        for pair in range(NPAIR + 1):
            tail = pair == NPAIR
            h0 = 2 * R * pair  # block A rows h0..h0+3, block B h0+4..h0+7
            xt = in_pool.tile([KDIM, 2 * WP], bf16, name="xt", tag="xt")
            nc.sync.dma_start(
                out=xt[:, 0:WP],
                in_=x[b, h0 * 12:(h0 + 8) * 12, :],
            )
            if not tail:
                nc.sync.dma_start(
                    out=xt[:, WP:2 * WP],
                    in_=x[b, (h0 + 4) * 12:(h0 + 12) * 12, :],
                )
            ps = psum_pool.tile([MDIM, 2 * WOP], f32, name="ps", tag="ps")
            for half in range(1 if tail else 2):
                for d in range(ND):
                    nc.tensor.matmul(
                        ps[:, half * WOP:(half + 1) * WOP],
                        tw[:, d * MDIM:(d + 1) * MDIM],
                        xt[:, half * WP + d:half * WP + d + WOP],
                        start=(d == 0),
                        stop=(d == ND - 1),
                    )
            ot = out_pool.tile([MDIM, 2 * WOP], bf16, name="ot", tag="ot")
            nc.vector.tensor_scalar_add(
                ot[:, 0:WOP], ps[:, 0:WOP], bias_sb[:, :])
            if tail:
                nc.scalar.dma_start(
                    out=y2[b, :, :, :], in_=ot[:, 0:WOP])
            else:
                nc.scalar.activation(
                    ot[:, WOP:2 * WOP], ps[:, WOP:2 * WOP], Ident,
                    bias=bias_sb[:, :])
                nc.scalar.dma_start(
                    out=y[b, pair, :, :, :], in_=ot[:, :])


def _prep_in_maps(x, w3, b3, w4, b4, w6, b6):
    from ml_dtypes import bfloat16

    x = np.asarray(x, dtype=np.float32)
    # de-interleave width and flatten: xp[b, (h, ci, s), j] = x[b,ci,h,S*j+s]
    xp = np.ascontiguousarray(
        x.reshape(32, 6, H, WP, S).transpose(0, 2, 1, 4, 3)
        .reshape(32, H * 6 * S, WP).astype(bfloat16))
    Wf = _build_full_kernel(np.asarray(w3, dtype=np.float32),
                            np.asarray(w4, dtype=np.float32),
                            np.asarray(w6, dtype=np.float32))
    T = np.ascontiguousarray(_build_toeplitz_packed(Wf).astype(bfloat16))
    bias16 = np.concatenate([np.asarray(b3, dtype=np.float32),
                             np.asarray(b4, dtype=np.float32),
                             np.asarray(b6, dtype=np.float32)])
    bias_col = np.ascontiguousarray(
        np.repeat(bias16, R * S)[:, None], dtype=np.float32)  # [co*8+r*2+p, 1]
    return [
        {"x": xp[i * B_PER_CORE:(i + 1) * B_PER_CORE], "t": T,
         "bias": bias_col}
        for i in range(N_CORES)
    ]


def _assemble_output(results):
    if PAIR:
        ym = np.concatenate([r["y"] for r in results], axis=0)
        yt = np.concatenate([r["y2"] for r in results], axis=0)
        # ym [b, pair, co, (r,p), (half,j)] -> [b, co, oh, ow]
        main = (ym.reshape(32, NPAIR, 16, R, S, 2, WOP)
                .transpose(0, 2, 1, 5, 3, 6, 4)
                .reshape(32, 16, NPAIR * 2 * R, WO))
        tail = (yt.reshape(32, 16, R, S, WOP)
                .transpose(0, 1, 2, 4, 3)
                .reshape(32, 16, R, WO))
        return np.concatenate([main, tail], axis=2)
    yp = np.concatenate([r["y"] for r in results], axis=0)
    # re-interleave: y[b,co,oh,S*j+p] = yp[b,co,oh*S+p,j]
    return (yp.reshape(32, 16, HO, S, WOP).transpose(0, 1, 2, 4, 3)
            .reshape(32, 16, HO, WO))


def kernel(x, w3, b3, w4, b4, w6, b6):
    global LAST_RESULTS
    from concourse.bass_utils import run_bass_kernel_spmd

    in_maps = _prep_in_maps(x, w3, b3, w4, b4, w6, b6)
    nc = _build_bass()
    res = run_bass_kernel_spmd(
        nc, in_maps, core_ids=list(range(N_CORES)),
        trace=bool(int(os.environ.get("CONV_TRACE", "0"))),
    )
    LAST_RESULTS = res
    out = _assemble_output(res.results)
    return np.ascontiguousarray(out.astype(np.float32))
